# revision 60
# baseline (speedup 1.0000x reference)
"""Trainium2 Bass kernel for ChunkTriangleAttentionStartingNode.

Computation (B=1, N=384, D=128, h=4, c=32):
  Z = LayerNorm(Z_raw) * ln_w + ln_b                     (over d_pair)
  bias[h,q,k]   = (Z @ W_b)[q,k,h]        (triangle bias, row-indexed by q)
  q,k,v         = split(Z @ W_qkv)        per pair-row i, heads h, dim c
  logits[i,h,q,k] = q.k / sqrt(c) + mask_bias[i,k] + bias[h,q,k]
  out = Z_raw + (sigmoid(Z@W_gate + gb) * softmax(logits) @ v) @ W_o + out_bias

Sharding: rows (first pair axis) split across 8 cores, 48 rows each; each
core computes its bias shard; ONE AllGather produces the full [h,N,N] bias.

Structure (~696us on HW, from a 751us graded baseline; per-trace analysis):
  - A dummy 64B AllGather fires at t=0 so the runtime's first-collective
    BARRIER (22-114us of core-start skew + CC warmup, run-variable) and the
    first-CC warmup run concurrently with phase 1 instead of serializing
    before the bias AllGather (v1 paid ~150us here).  Collective triggers
    BLOCK the issuing gpsimd queue until completion, so gpsimd carries ONLY
    [dummy CC, real CC, per-row accum DMAs] and constants are loaded via
    inline tensors (no gpsimd memsets ahead of the CC).
  - Phase 1 in two passes so the single batched Sqrt (stats finish) runs
    before any scheduler-hoisted front exp can thrash ACT table sets:
    pass A streams rows through bn_stats (3 per row; the interp mis-indexes
    multi-group outputs), pass B re-DMAs rows and normalizes (one chunk on
    ACT Identity, two on DVE), PE-transposes into resident bf16 Z^T,
    projects the bias shard and DMAs it out.
  - ONE bias AllGather (147KB in / 1.2MB out, ~15-50us run-variable)
    instead of 3 chunked ones (3x17us + 2x12us queue gaps).
  - exp(bias^T) tables: 4 head transposes packed into one PSUM bank ->
    one [128,512] exp per (kc, dev-chunk): 9 ACT ops instead of 36.
  - Rows split front/back: front(i) = projections+logits+exp(masked)
    (needs no bias), back(i) = exp(bias) multiply onward.  PRE=5 fronts
    are emitted before the Eb tables to fill the AllGather wait; then
    back(j) and front(j+PRE) interleave so back-heavy DVE overlaps
    front-heavy ACT.  PRE=2 starves the pipeline (12.1us/row vs 9.8).
  - exp(l+b) = exp(l)*exp(b): the softmax bias-add is one [128,4,384] bf16
    DVE multiply per (row, kc) against the resident Eb tables.
  - Residual + out_bias via DMA/PE: out_bias is preloaded into the out-proj
    PSUM by a K=1 matmul (start=True sets has_written; projections
    accumulate on top), att = one PSUM->SBUF copy, then a gpsimd CCE DMA
    accumulates Z_raw[i] into att (accum_op=add) and a sync DMA stores it.
    Removes v1's zrow2 reload and 4-op DVE residual chain.
  - QK^T computed transposed ([k,q]) with 4 heads packed via tile_position
    row bands (concurrent); softmax sums via 2.0-valued ones matmuls
    col-packed per head (factor 2 absorbs the 0.5 of
    sigmoid(x)=(1+tanh(x/2))/2); normalization by reciprocal_approx_fast;
    output projection uses gwa chunks as stationary producing [tok,d].
  - PSUM (8 banks): bigA/bigB 2x2 (lg halves alternate; pjB+v on bigB),
    acc 2 (wap+sums accumulators), pjo 2 (pjA ring-shared with out_ps).
  - Known wall (measured): phase 2 paces at ~9.8us/row because the PE runs
    HAM-throttled at 1.2GHz (K=4/8) nearly all of phase 2 -- MMs at the
    isolated-cold ~500ns instead of ~160ns warm; HAM grants K=8/8 for one
    3.4us window then revokes.  Heater matmuls into spare PSUM columns made
    it worse (LDW + FIFO cost > clock gain).  With a warm PE the same
    schedule would pace at ~max(ACT 6.0, DVE 6.3)us/row.
"""

import os
import sys

os.environ.setdefault("NEURON_RT_RESET_CORES", "1")

for _p in ("/opt/trn_rl_repo",):
    if _p not in sys.path:
        sys.path.append(_p)

import numpy as np
import ml_dtypes

import concourse.bass as bass
import concourse.bacc as bacc
import concourse.tile as tile
from concourse import mybir

F32 = mybir.dt.float32
BF16 = mybir.dt.bfloat16
AF = mybir.ActivationFunctionType
ALU = mybir.AluOpType
AX = mybir.AxisListType

# incremental-feature flags for bisection
F_DUMMYCC = os.environ.get("K_DUMMYCC", "1") == "1"   # absorb CC barrier at t=0
F_ONEAG = os.environ.get("K_ONEAG", "1") == "1"       # single bias AllGather
F_CCRES = os.environ.get("K_CCRES", "1") == "1"       # residual via CCE DMA accum
# wm-mul granularity per (row, kc): 3 = one [P,4,N] op, 6 = two [P,2,N],
# 12 = four [P,N] ops
WM_MODE = int(os.environ.get("K_WM", "3"))
F_F32T = os.environ.get("K_F32T", "0") == "1"         # f32 normalize+transpose
F_HEAT = os.environ.get("K_HEAT", "0") == "1"         # PE heater matmuls (HAM)
PRE = int(os.environ.get("K_PRE", "5"))               # fronts before Eb tables

P = 128          # partitions
D = 128          # d_pair
NH = 4           # heads
CH = 32          # head dim
HC = NH * CH     # 128


def build_nc(N=384, n_cores=8):
    C3 = N // P           # chunks along the attention axis
    R = N // n_cores      # rows per core
    SGB = 8               # zrow pool depth (stats and norm passes stream)
    pre = min(PRE, R)

    nc = bacc.Bacc(
        "TRN2",
        target_bir_lowering=False,
        debug=False,
        enable_asserts=False,
        num_devices=n_cores,
    )

    Zr = nc.dram_tensor("z_raw", [R, N, D], F32, kind="ExternalInput").ap()
    Zm = nc.dram_tensor("z_mask", [R, N], F32, kind="ExternalInput").ap()
    lnw_d = nc.dram_tensor("ln_w", [D], F32, kind="ExternalInput").ap()
    lnb_d = nc.dram_tensor("ln_b", [D], F32, kind="ExternalInput").ap()
    wb_d = nc.dram_tensor("w_b", [D, NH], F32, kind="ExternalInput").ap()
    wqkv_d = nc.dram_tensor("w_qkv", [D, 3 * HC], F32, kind="ExternalInput").ap()
    wg_d = nc.dram_tensor("w_gate", [D, HC], F32, kind="ExternalInput").ap()
    gb_d = nc.dram_tensor("gating_bias", [HC], F32, kind="ExternalInput").ap()
    wo_d = nc.dram_tensor("w_o", [HC, D], F32, kind="ExternalInput").ap()
    ob_d = nc.dram_tensor("out_bias", [D], F32, kind="ExternalInput").ap()
    OUT = nc.dram_tensor("out", [R, N, D], F32, kind="ExternalOutput").ap()

    id_bf_d = nc.inline_tensor(np.eye(P, dtype=ml_dtypes.bfloat16), "id_bf_c").ap()
    id_f_d = nc.inline_tensor(np.eye(P, dtype=np.float32), "id_f_c").ap()
    ones_d = nc.inline_tensor(
        np.full((P, CH), 2.0, dtype=ml_dtypes.bfloat16), "ones_c"
    ).ap()
    eps_d = nc.inline_tensor(np.full((P, 1), 1e-5, dtype=np.float32), "eps_c").ap()
    neg1e9_d = nc.inline_tensor(
        np.full((P, 1), -1e9, dtype=np.float32), "n1e9_c"
    ).ap()
    ones1_d = nc.inline_tensor(np.ones((1, P), dtype=np.float32), "ones1_c").ap()
    zeros16_d = nc.inline_tensor(np.zeros(16, dtype=np.float32), "zeros16_c").ap()

    with tile.TileContext(nc) as tc:
        with (
            tc.tile_pool(name="const", bufs=1) as constp,
            tc.tile_pool(name="res", bufs=1) as resp,
            tc.tile_pool(name="work", bufs=3) as work,
            tc.tile_pool(name="stat", bufs=4) as statp,
            tc.tile_pool(name="wpool", bufs=4) as wpool,
            tc.tile_pool(name="stash", bufs=3 * (pre + 2)) as stashp,
            tc.tile_pool(name="vstash", bufs=pre + 2) as vstashp,
            tc.tile_pool(name="attp", bufs=4) as attp,
            tc.tile_pool(name="zpool", bufs=SGB) as zpool,
            tc.tile_pool(name="ps", bufs=1, space="PSUM") as psum,
            tc.tile_pool(name="dram", bufs=1, space="DRAM") as dramp,
        ):
            # DRAM tensors for the collectives
            b_shard = dramp.tile([R, NH, N], BF16, tag="bshard")
            if F_ONEAG:
                b_fulls = [
                    dramp.tile(
                        [n_cores, R, NH, N], BF16, tag="bfull0",
                        addr_space="Shared", name="bfull0",
                    )
                ]
                ag_bounds = [(0, R)]
            else:
                ag_edges = [R // 3, (2 * R) // 3, R]
                ag_bounds = [(0 if i == 0 else ag_edges[i - 1], ag_edges[i])
                             for i in range(3)]
                b_fulls = [
                    dramp.tile(
                        [n_cores, hi - lo, NH, N], BF16, tag=f"bfull{i}",
                        addr_space="Shared", name=f"bfull{i}",
                    )
                    for i, (lo, hi) in enumerate(ag_bounds)
                ]

            if F_DUMMYCC:
                agd_in = dramp.tile([16], F32, tag="agdin")
                nc.sync.dma_start(agd_in, zeros16_d)
                agd_out = dramp.tile(
                    [n_cores, 16], F32, tag="agdout",
                    addr_space="Shared", name="agdout",
                )
                nc.gpsimd.collective_compute(
                    "AllGather",
                    ALU.bypass,
                    replica_groups=[list(range(n_cores))],
                    ins=[agd_in.opt()],
                    outs=[agd_out.opt()],
                )

            # ---- constants / weights ----
            id_bf = constp.tile([P, P], BF16)
            nc.sync.dma_start(id_bf, id_bf_d)
            if F_F32T:
                id_f = constp.tile([P, P], F32)
                nc.sync.dma_start(id_f, id_f_d)
            else:
                id_f = None
            ones_bf = constp.tile([P, CH], BF16)
            nc.sync.dma_start(ones_bf, ones_d)
            eps_c = constp.tile([P, 1], F32)
            nc.sync.dma_start(eps_c, eps_d)
            neg1e9_c = constp.tile([P, 1], F32)
            nc.sync.dma_start(neg1e9_c, neg1e9_d)
            ones1 = constp.tile([1, P], F32)
            nc.sync.dma_start(ones1, ones1_d)

            lnw = constp.tile([D, 1], F32)
            nc.sync.dma_start(lnw, lnw_d[:, None])
            lnb = constp.tile([D, 1], F32)
            nc.sync.dma_start(lnb, lnb_d[:, None])
            gb = constp.tile([HC, 1], F32)
            nc.sync.dma_start(gb, gb_d[:, None])
            ngb = constp.tile([HC, 1], F32)
            nc.scalar.mul(ngb, gb, 0.5)

            wtmp = constp.tile([D, 3 * HC], F32, tag="wtmp")
            nc.sync.dma_start(wtmp, wqkv_d)
            wq = constp.tile([D, HC], BF16)
            nc.scalar.activation(wq, wtmp[:, 0:HC], AF.Copy, scale=CH ** -0.5)
            wk = constp.tile([D, HC], BF16)
            nc.scalar.copy(wk, wtmp[:, HC:2 * HC])
            wv = constp.tile([D, HC], BF16)
            nc.scalar.copy(wv, wtmp[:, 2 * HC:3 * HC])

            wgt = constp.tile([D, HC], F32, tag="wgt")
            nc.sync.dma_start(wgt, wg_d)
            wg = constp.tile([D, HC], BF16)
            nc.scalar.copy(wg, wgt)
            wot = constp.tile([HC, D], F32, tag="wot")
            nc.sync.dma_start(wot, wo_d)
            wo = constp.tile([HC, D], BF16)
            nc.scalar.copy(wo, wot)
            wbt = constp.tile([D, NH], F32, tag="wbt")
            nc.sync.dma_start(wbt, wb_d)
            wb = constp.tile([D, NH], BF16)
            nc.scalar.copy(wb, wbt)

            # out_bias row [1, (c,d)]: preloaded into the out-proj PSUM via a
            # K=1 matmul so the residual stage needs only a PSUM->SBUF copy
            obr3 = constp.tile([1, C3, D], F32)
            for c in range(C3):
                nc.sync.dma_start(obr3[:, c, :], ob_d[None, :])
            obr3f = obr3.rearrange("one c d -> one (c d)")

            # mask bias columns: mb[kc][k, i] = (Z_mask[i, k] - 1) * 1e9
            mb = []
            for kc in range(C3):
                mk = work.tile([P, R], F32, tag="mk")
                nc.sync.dma_start(
                    mk, Zm[:, kc * P:(kc + 1) * P].rearrange("r p -> p r")
                )
                mbt = resp.tile([P, R], F32, tag=f"mb{kc}", name=f"mb{kc}")
                nc.scalar.activation(mbt, mk, AF.Identity, scale=1e9, bias=neg1e9_c)
                mb.append(mbt)

            # ---- phase 1: LayerNorm -> resident Z^T, bias shard ----
            # Two passes: (A) stats only (s1 on DVE reduce, s2 on ACT
            # square+accum -- Square is in every ACT table set so the
            # single batched Sqrt below is the only table at risk, and it
            # runs before any front exp becomes eligible), then (B)
            # normalize+transpose+bias with a second zrow DMA.
            rsig_all = resp.tile([P, R, C3], F32, tag="rsig_all")
            nmr_all = resp.tile([P, R, C3], F32, tag="nmr_all")
            Zt = resp.tile([P, R * C3 * P], BF16, tag="Zt")

            # zrow loads alternate between the two HWDGE rings (sync and
            # scalar paths) -- a single ring FIFO-serializes the 48 DMAs at
            # ~1.9us each and made the stats pass DMA-latency-bound
            def zrow_dma(q, zrow):
                eng = nc.sync if q % 2 == 0 else nc.scalar
                eng.dma_start(zrow, Zr[q].rearrange("(c p) d -> p c d", p=P))

            bst_all = resp.tile([P, R, C3, 6], F32, tag="bst_all")
            for q in range(R):
                zrow = zpool.tile([P, C3, P], F32, tag="zrow")
                zrow_dma(q, zrow)
                for c in range(C3):
                    nc.vector.bn_stats(bst_all[:, q, c, :], zrow[:, c, :])

            n = R * C3
            rsg = rsig_all.rearrange("p r c -> p (r c)")
            nmg = nmr_all.rearrange("p r c -> p (r c)")
            me = bst_all[:, :, :, 1:2].rearrange("p r c one -> p (r c one)")
            mo = bst_all[:, :, :, 4:5].rearrange("p r c one -> p (r c one)")
            cve = bst_all[:, :, :, 2:3].rearrange("p r c one -> p (r c one)")
            cvo = bst_all[:, :, :, 5:6].rearrange("p r c one -> p (r c one)")
            msum = statp.tile([P, n], F32, tag="msum")
            nc.vector.tensor_add(msum, me, mo)
            dm = statp.tile([P, n], F32, tag="dm")
            nc.vector.tensor_sub(dm, me, mo)
            dmh = statp.tile([P, n], F32, tag="dmh")
            nc.vector.tensor_scalar_mul(dmh, dm, 0.5)
            dm2 = statp.tile([P, n], F32, tag="dm2")
            nc.vector.tensor_mul(dm2, dmh, dmh)
            cvs = statp.tile([P, n], F32, tag="cvs")
            nc.vector.tensor_add(cvs, cve, cvo)
            # var = (cve+cvo)/D + ((me-mo)/2)^2 ; cvX are count*var halves
            v3t = statp.tile([P, n], F32, tag="v3t")
            nc.vector.scalar_tensor_tensor(
                v3t, cvs, 1.0 / D, dm2, op0=ALU.mult, op1=ALU.add
            )
            stdg = statp.tile([P, n], F32, tag="stdg")
            nc.scalar.activation(stdg, v3t, AF.Sqrt, bias=eps_c)
            nc.vector.reciprocal(rsg, stdg)
            nc.vector.scalar_tensor_tensor(
                nmg, msum, -0.5, rsg, op0=ALU.mult, op1=ALU.mult
            )

            tdt = F32 if F_F32T else BF16
            tid = id_f if F_F32T else id_bf

            for q in range(R):
                zrow = zpool.tile([P, C3, P], F32, tag="zrow")
                zrow_dma(q, zrow)
                tpr = psum.tile([P, 2, 512], tdt, tag="pjo", bufs=1, name="tpr")
                tp = tpr[:, 0, 0:C3 * P].rearrange("p (c q2) -> p c q2", c=C3)
                for c in range(C3):
                    zn = work.tile([P, P], tdt, tag="zn")
                    if c == 0:
                        # one chunk on ACT (Identity is in every table set)
                        # to balance the DVE-bound normalize pass
                        nc.scalar.activation(
                            zn, zrow[:, c, :], AF.Identity,
                            scale=rsig_all[:, q, c:c + 1],
                            bias=nmr_all[:, q, c:c + 1],
                        )
                    else:
                        nc.vector.tensor_scalar(
                            zn, zrow[:, c, :],
                            rsig_all[:, q, c:c + 1], nmr_all[:, q, c:c + 1],
                            op0=ALU.mult, op1=ALU.add,
                        )
                    nc.tensor.transpose(tp[:, c, :], zn, tid)
                nc.vector.tensor_scalar(
                    Zt[:, q * C3 * P:(q + 1) * C3 * P].rearrange(
                        "p (c q2) -> p c q2", c=C3
                    ),
                    tp, lnw, lnb, op0=ALU.mult, op1=ALU.add,
                )
                bpr = psum.tile([P, 2, 512], F32, tag="acc", bufs=1, name="bpr")
                bp = bpr[0:NH, 0, 0:N]
                nc.tensor.matmul(bp, wb, Zt[:, q * C3 * P:(q + 1) * C3 * P])
                bsb = work.tile([NH, N], BF16, tag="bsb")
                nc.scalar.copy(bsb, bp)
                (nc.sync if q % 2 == 0 else nc.scalar).dma_start(
                    b_shard[q], bsb
                )
                for idx, (lo, hi) in enumerate(ag_bounds):
                    if q + 1 == hi:
                        nc.gpsimd.collective_compute(
                            "AllGather",
                            ALU.bypass,
                            replica_groups=[list(range(n_cores))],
                            ins=[b_shard[lo:hi].opt()],
                            outs=[b_fulls[idx].opt()],
                        )

            # exp of transposed bias, resident per k-chunk: Eb[kc][k, h, q].
            Eb = [
                resp.tile([P, NH, N], BF16, tag=f"eb{kc}", name=f"eb{kc}")
                for kc in range(C3)
            ]

            # ---- phase 2 per-row pieces ----
            wH = {}      # (i, kc) -> [P, NH, N] bf16 stash
            vsbs = [None] * R
            ths = [None] * R

            qks = [None] * R

            def lg_mm(i, kc, half, qk_sb):
                qt = qk_sb[:, 0, :]
                kt = qk_sb[:, 1, :]
                lgH = psum.tile(
                    [P, 2, 512], F32, tag="bigA" if half == 0 else "bigB",
                    bufs=1, name=f"lg{kc}{half}",
                )
                for hh in range(2):
                    h = half * 2 + hh
                    nc.tensor.matmul(
                        lgH[:, hh, 0:N],
                        kt[CH * h:CH * (h + 1), kc * P:(kc + 1) * P],
                        qt[CH * h:CH * (h + 1), :],
                        tile_position=(CH * h, 0),
                    )
                return lgH

            def lg_exp(i, kc, half, lgH):
                nc.scalar.activation(
                    wH[(i, kc)][:, 2 * half:2 * half + 2, :],
                    lgH[:, :, 0:N], AF.Exp, bias=mb[kc][:, i:i + 1]
                )

            def lg_half(i, kc, qk_sb):
                # both halves' matmuls issued adjacently: 4 distinct PE row
                # bands run concurrently before the two exps drain them
                lg0 = lg_mm(i, kc, 0, qk_sb)
                lg1 = lg_mm(i, kc, 1, qk_sb)
                lg_exp(i, kc, 0, lg0)
                lg_exp(i, kc, 1, lg1)

            def front_head(i):
                zt_row = Zt[:, i * C3 * P:(i + 1) * C3 * P]
                # PSUM ring order per steady iteration: pjA, lg00, lg01
                # (head), then back(i-pre), then pjB, lg10..lg21 (tail) --
                # the next row's first exps are always produced before ACT
                # drains the current row's, keeping the exp stream gapless.
                for kc in range(C3):
                    wHt = stashp.tile([P, NH, N], BF16, tag="wh", name="wHt")
                    wH[(i, kc)] = wHt
                pjA = psum.tile([P, 2, 512], F32, tag="pjo", bufs=1, name="pjA")
                nc.tensor.matmul(pjA[:, 0, 0:N], wq, zt_row)
                nc.tensor.matmul(pjA[:, 1, 0:N], wk, zt_row)
                qk_sb = work.tile([P, 2, N], BF16, tag="qk_sb", bufs=4)
                nc.vector.tensor_copy(qk_sb, pjA[:, :, 0:N])
                qks[i] = qk_sb
                lg_half(i, 0, qk_sb)

            def front_tail(i):
                zt_row = Zt[:, i * C3 * P:(i + 1) * C3 * P]
                qk_sb = qks[i]
                pjB = psum.tile([P, 2, 512], F32, tag="bigB", bufs=1, name="pjB")
                nc.tensor.matmul(pjB[:, 0, 0:N], wg, zt_row)
                for c in range(C3):
                    nc.tensor.matmul(
                        pjB[:, 1, c * P:(c + 1) * P],
                        zt_row[:, c * P:(c + 1) * P],
                        wv,
                    )
                th = vstashp.tile([P, N], BF16, tag="th")
                nc.scalar.activation(th, pjB[:, 0, 0:N], AF.Tanh, scale=0.5, bias=ngb)
                ths[i] = th
                vsb3 = vstashp.tile([P, C3, P], BF16, tag="vsb")
                nc.vector.tensor_copy(
                    vsb3.rearrange("p c q2 -> p (c q2)"), pjB[:, 1, 0:N]
                )
                vsbs[i] = vsb3
                for kc in range(1, C3):
                    lg_half(i, kc, qk_sb)
                qks[i] = None

            def row_back(i):
                wap3 = psum.tile([P, 2, 512], F32, tag="acc", bufs=1, name="wap3")
                wap = wap3[:, 0, 0:N]
                sp = wap3[:, 1, 0:N]
                vsb3 = vsbs[i]

                def heat(outsl, ncols, tag):
                    # keep PE_HAM's activity window busy so the PE clock
                    # stays at 2.4 GHz; start=False never clears the bank
                    nc.tensor.matmul(
                        outsl, id_bf,
                        Zt[:, i * C3 * P:i * C3 * P + ncols],
                        start=False, stop=False, skip_group_check=True,
                    )
                for kc in range(C3):
                    wHt = wH.pop((i, kc))
                    wms = []
                    if WM_MODE == 3:
                        wmA = wpool.tile([P, NH, N], BF16, tag="wmA")
                        nc.vector.tensor_mul(wmA, wHt, Eb[kc])
                        wms = [wmA[:, h, :] for h in range(NH)]
                    elif WM_MODE == 6:
                        for half in (0, 1):
                            wmH = wpool.tile([P, 2, N], BF16, tag=f"wm{half}")
                            nc.vector.tensor_mul(
                                wmH, wHt[:, 2 * half:2 * half + 2, :],
                                Eb[kc][:, 2 * half:2 * half + 2, :]
                            )
                            wms.extend([wmH[:, 0, :], wmH[:, 1, :]])
                    else:
                        for h in range(NH):
                            wm = wpool.tile([P, N], BF16, tag="wm")
                            nc.vector.tensor_mul(
                                wm, wHt[:, h, :], Eb[kc][:, h, :]
                            )
                            wms.append(wm)
                    for h in range(NH):
                        nc.tensor.matmul(
                            wap[CH * h:CH * (h + 1), :],
                            vsb3[:, kc, CH * h:CH * (h + 1)],
                            wms[h],
                            start=(kc == 0),
                            stop=(kc == C3 - 1),
                            skip_group_check=True,
                            tile_position=(0, CH * h),
                        )
                    for h in range(NH):
                        nc.tensor.matmul(
                            sp[CH * h:CH * (h + 1), :],
                            ones_bf,
                            wms[h],
                            start=(kc == 0),
                            stop=(kc == C3 - 1),
                            skip_group_check=True,
                            tile_position=(0, CH * h),
                        )
                    if F_HEAT:
                        heat(wap3[:, 0, N:512], 512 - N, "h0")
                        heat(wap3[:, 1, N:512], 512 - N, "h1")

                rs = work.tile([P, N], F32, tag="rs")
                nc.vector.reciprocal_approx_fast(rs, sp)
                wan = work.tile([P, N], F32, tag="wan")
                nc.vector.tensor_mul(wan, wap, rs)
                gwa = work.tile([P, N], BF16, tag="gwa")
                nc.vector.scalar_tensor_tensor(
                    gwa, ths[i], 1.0, wan, op0=ALU.add, op1=ALU.mult
                )
                ths[i] = None

                opr = psum.tile([P, 2, 512], F32, tag="pjo", bufs=1, name="opr")
                out_flat = opr[:, 0, 0:C3 * P]
                out_ps = out_flat.rearrange("p (c q2) -> p c q2", c=C3)
                # preload the broadcast out_bias row, then accumulate the
                # projection on top (K=1 matmul sets has_written)
                nc.tensor.matmul(
                    out_flat, ones1, obr3f,
                    start=True, stop=False, skip_group_check=True,
                )
                for c in range(C3):
                    nc.tensor.matmul(
                        out_ps[:, c, :], gwa[:, c * P:(c + 1) * P], wo,
                        start=False, stop=(c == C3 - 1), skip_group_check=True,
                    )
                if F_HEAT:
                    nc.tensor.matmul(
                        opr[:, 1, 0:P], id_bf,
                        Zt[:, i * C3 * P:i * C3 * P + P],
                        start=True, stop=False, skip_group_check=True,
                    )
                    heat(opr[:, 1, P:2 * P], P, "h3")
                att = attp.tile([P, C3, P], F32, tag="att")
                nc.vector.tensor_copy(
                    att.rearrange("p c q2 -> p (c q2)"), out_flat
                )
                if F_CCRES:
                    nc.gpsimd.dma_start(
                        att, Zr[i].rearrange("(c p) d -> p c d", p=P),
                        accum_op=ALU.add,
                    )
                    nc.sync.dma_start(
                        OUT[i].rearrange("(c p) d -> p c d", p=P), att
                    )
                else:
                    zrow2 = work.tile([P, C3, P], F32, tag="zrow2")
                    nc.sync.dma_start(
                        zrow2, Zr[i].rearrange("(c p) d -> p c d", p=P)
                    )
                    fin = work.tile([P, C3, P], F32, tag="fin")
                    nc.vector.tensor_add(fin, att, zrow2)
                    nc.sync.dma_start(
                        OUT[i].rearrange("(c p) d -> p c d", p=P), fin
                    )

            # fronts that fill the AllGather wait
            for i in range(pre):
                front_head(i)
                front_tail(i)

            # Eb tables (gated on the AllGather via the bt DMAs)
            eb_edges = [R // 3, (2 * R) // 3, R]
            eb_bounds = [(0 if i == 0 else eb_edges[i - 1], eb_edges[i])
                         for i in range(3)]
            bt = []
            for c, (lo, hi) in enumerate(eb_bounds):
                npart = n_cores * (hi - lo)
                btq = resp.tile([npart, NH, N], BF16, tag=f"bt{c}", name=f"bt{c}")
                if F_ONEAG:
                    nr = hi - lo
                    for dv in range(n_cores):
                        (nc.sync if dv % 2 == 0 else nc.scalar).dma_start(
                            btq[dv * nr:(dv + 1) * nr],
                            b_fulls[0][dv, lo:hi],
                        )
                else:
                    nc.sync.dma_start(
                        btq, b_fulls[c].rearrange("dev r h k -> (dev r) h k")
                    )
                bt.append(btq)

            for kc in range(C3):
                for c, (lo, hi) in enumerate(eb_bounds):
                    nr = hi - lo
                    npart = n_cores * nr
                    # 4 head transposes into one PSUM bank -> one wide exp
                    tp2r = psum.tile(
                        [P, 2, 512], BF16, tag="bigA", bufs=1, name="tp2r"
                    )
                    tp4 = tp2r.rearrange("p b (two q2) -> p (b two) q2", two=2)
                    for h in range(NH):
                        nc.tensor.transpose(
                            tp4[:, h, 0:npart],
                            bt[c][:, h, kc * P:(kc + 1) * P],
                            id_bf[0:npart, 0:npart],
                        )
                    nc.scalar.activation(
                        Eb[kc].rearrange(
                            "p h (dev r) -> p h dev r", dev=n_cores
                        )[:, :, :, lo:hi],
                        tp4[:, :, 0:npart].rearrange(
                            "p h (dev r) -> p h dev r", dev=n_cores
                        ),
                        AF.Exp,
                    )

            # steady state: back(j) overlaps front(j+pre); pjA/out_ps live in
            # their own PSUM pool so projections run a full row ahead
            for j in range(R):
                row_back(j)
                if j + pre < R:
                    front_head(j + pre)
                    front_tail(j + pre)

    nc.compile()
    return nc


_CACHE = {}


def get_nc(N=384, n_cores=8):
    key = (N, n_cores)
    if key not in _CACHE:
        _CACHE[key] = build_nc(N, n_cores)
    return _CACHE[key]


def make_in_maps(inputs, N=384, n_cores=8):
    R = N // n_cores
    Z = np.ascontiguousarray(np.asarray(inputs["Z_raw"], dtype=np.float32))
    M = np.ascontiguousarray(np.asarray(inputs["Z_mask"], dtype=np.float32))
    shared = {
        "ln_w": np.ascontiguousarray(np.asarray(inputs["ln_w"], np.float32)),
        "ln_b": np.ascontiguousarray(np.asarray(inputs["ln_b"], np.float32)),
        "w_b": np.ascontiguousarray(np.asarray(inputs["W_b"], np.float32)),
        "w_qkv": np.ascontiguousarray(np.asarray(inputs["W_qkv"], np.float32)),
        "w_gate": np.ascontiguousarray(np.asarray(inputs["W_gate"], np.float32)),
        "gating_bias": np.ascontiguousarray(
            np.asarray(inputs["gating_bias"], np.float32)
        ),
        "w_o": np.ascontiguousarray(np.asarray(inputs["W_o"], np.float32)),
        "out_bias": np.ascontiguousarray(np.asarray(inputs["out_bias"], np.float32)),
    }
    in_maps = []
    for c in range(n_cores):
        m = dict(shared)
        m["z_raw"] = np.ascontiguousarray(Z[0, c * R:(c + 1) * R])
        m["z_mask"] = np.ascontiguousarray(M[0, c * R:(c + 1) * R])
        in_maps.append(m)
    return in_maps


def kernel(**inputs):
    from concourse.bass_utils import run_bass_kernel_spmd

    N, n_cores = 384, 8
    nc = get_nc(N, n_cores)
    in_maps = make_in_maps(inputs, N, n_cores)
    res = run_bass_kernel_spmd(nc, in_maps, list(range(n_cores)))
    out = np.concatenate([res.results[c]["out"] for c in range(n_cores)], axis=0)
    return out.reshape(1, N, N, D).astype(np.float32)
